# revision 8
# baseline (speedup 1.0000x reference)
"""Trainium2 Bass kernel for nn_CNN2DAttWind_NL (conv + 32-head rank-1 attention + MLP).

Contract: kernel(**inputs) takes FULL unsharded numpy inputs, returns FULL output
[8192, 5] float32.  Internally shards the batch across 8 NeuronCores (pure data
parallel) and runs a Bass/Tile kernel via run_bass_kernel_spmd.

Layout strategy per core (B_local = 1024), chunk CB = 128 samples:
  - conv as matmul with the BIAS FOLDED IN as a 17th K-row (moving row of
    ones); run per b-quarter g so the eviction writes q/k/v straight into the
    pair-major attention layout [(g, head), b', pix] (no SBUF shuffle DMAs).
  - attention: partitions = (b-quarter, head) = 128 rows; free = (b', i, j).
    z = q*k on DVE/Pool (alternating subs), exp on ACT, Ev on DVE, then
    two-stage pair-adds (TT at 2x mode) + a 7-wide reduce for num/den
    (tensor_reduce is 1 elem/cycle, so feeding it 7/25th of the data wins).
  - 1x1 conv as a block-diagonal [128,128] stationary consuming attn in
    pair-major directly; 3-layer MLP as PE matmuls.
"""

import os
from contextlib import ExitStack

import numpy as np

# Problem constants (hardcoded per harness contract)
B, CIN, HIN, WIN = 8192, 16, 6, 6
FM = 64
DK = FM // 2          # 32
DV = FM // 2          # 32
NH = FM // 2          # 32 heads, dkh = dvh = 1
OUT_CH = 5
HID = 128
HW = 25               # 5x5 output pixels

NCORES = 8
BL = B // NCORES      # 1024 samples per core
CB = 128              # samples per outer chunk
NCHUNK = BL // CB     # 8
BP = CB // 4          # 32 b' per quarter
SB = 8                # b' groups per attention sub-chunk
NSUB = BP // SB       # 4
VS = 16.0             # v pre-scale to keep Ev pair-adds inside f16 range


def _build_program():
    import concourse.bass as bass
    import concourse.tile as tile
    from concourse import bacc, mybir

    f32 = mybir.dt.float32
    f16 = mybir.dt.float16
    AF = mybir.ActivationFunctionType
    ALU = mybir.AluOpType
    AX = mybir.AxisListType

    nc = bacc.Bacc(
        "TRN2",
        target_bir_lowering=False,
        debug=False,
        enable_asserts=False,
        num_devices=NCORES,
    )

    # ---- DRAM I/O ----
    x_d = nc.dram_tensor("x_s", [BL, CIN, HIN, WIN], f16, kind="ExternalInput")
    wp32_d = nc.dram_tensor("wp32", [128, 4], f32, kind="ExternalInput")
    wp16_d = nc.dram_tensor("wp16", [128, 4000], f16, kind="ExternalInput")
    out_d = nc.dram_tensor("out", [5, BL], f32, kind="ExternalOutput")

    in_names = [t.name for t in (x_d, wp32_d, wp16_d)]

    with tile.TileContext(nc) as tc, ExitStack() as ctx, \
            nc.allow_low_precision(reason="f16 softmax intermediates, 2e-2 budget"):
        singles = ctx.enter_context(tc.tile_pool(name="singles", bufs=1))
        xa_p = ctx.enter_context(tc.tile_pool(name="xa", bufs=2))
        qkv_p = ctx.enter_context(tc.tile_pool(name="qkv", bufs=2))
        yt_p = ctx.enter_context(tc.tile_pool(name="yt", bufs=2))
        att_p = ctx.enter_context(tc.tile_pool(name="att", bufs=2))
        z_p = ctx.enter_context(tc.tile_pool(name="z", bufs=2))
        t2_p = ctx.enter_context(tc.tile_pool(name="t2", bufs=2))
        red_p = ctx.enter_context(tc.tile_pool(name="red", bufs=3))
        mlp_p = ctx.enter_context(tc.tile_pool(name="mlp", bufs=2))
        outp_p = ctx.enter_context(tc.tile_pool(name="outp", bufs=2))
        ps_conv = ctx.enter_context(tc.tile_pool(name="psc", bufs=3, space="PSUM"))
        ps_att = ctx.enter_context(tc.tile_pool(name="psa", bufs=2, space="PSUM"))
        ps_mlp = ctx.enter_context(tc.tile_pool(name="psm", bufs=1, space="PSUM"))

        # ---- load all weights in 2 DMAs ----
        wp32 = singles.tile([128, 4], f32)
        nc.sync.dma_start(out=wp32, in_=wp32_d.ap())
        wp16 = singles.tile([128, 4000], f16)
        nc.sync.dma_start(out=wp16, in_=wp16_d.ap())
        expb_s = wp32[:, 0:1]
        b1_s = wp32[:, 1:2]
        b2_s = wp32[0:64, 2:3]
        ab_s = wp32[0:32, 3:4]
        w2_s = wp16[:, 0:64]
        w1_s = wp16[0:64, 64:3264].rearrange("p (h i) -> p h i", i=25)
        aw_s = wp16[:, 3264:3392]                # block-diag 1x1 conv [128,128]
        w3_s = wp16[0:64, 3392:3397]
        wconv_s = wp16[0:17, 3397:3909].rearrange("p (g c) -> p g c", g=4)

        # ---- persistent attention tiles (pad columns pre-zeroed once) ----
        # E/Ev are 26 wide: cols 0:25 written each use, col 25 stays 0.
        # T1 are 14 wide: cols 0:13 written each use, col 13 stays 0.
        E_b = [singles.tile([128, SB, 25, 28], f16, name=f"E{i}") for i in range(2)]
        Ev_b = [singles.tile([128, SB, 25, 28], f16, name=f"Ev{i}") for i in range(2)]
        T1n_b = [singles.tile([128, SB, 25, 16], f16, name=f"T1n{i}") for i in range(2)]
        T1d_b = [singles.tile([128, SB, 25, 16], f16, name=f"T1d{i}") for i in range(2)]
        for t in E_b + Ev_b:
            nc.vector.memset(t[:, :, :, 25:28], 0.0)
        for t in T1n_b + T1d_b:
            nc.vector.memset(t[:, :, :, 14:16], 0.0)
        # xa double buffer: rows 0:16 = x, row 16 = ones (bias path)
        xa_b = [singles.tile([32, CB, 36], f16, name=f"xa{i}") for i in range(2)]
        for t in xa_b:
            nc.vector.memset(t, 1.0)

        for c in range(NCHUNK):
            b0 = c * CB
            xa = xa_b[c % 2]
            src = bass.AP(tensor=x_d, offset=b0 * 576,
                          ap=[[36, 16], [576, CB], [1, 36]])
            nc.sync.dma_start(out=xa[0:16], in_=src)

            # ---- conv per quarter: psum rows = q(0:32) k v conv_out ----
            q_d = qkv_p.tile([128, BP, 25, 2], f16, tag="q_d")
            k_t = qkv_p.tile([128, BP, 26], f16, tag="k_t")
            v_t = qkv_p.tile([128, BP, 26], f16, tag="v_t")
            y_t = yt_p.tile([64, CB, 25], f16, tag="y_t")
            for g in range(4):
                for t0 in range(2):             # two 400-col tiles of 16 b'
                    s0 = 32 * g + 16 * t0       # first sample of tile
                    bp0 = 16 * t0               # first b' of tile
                    pscv = ps_conv.tile([128, 400], f32)
                    for dydx in range(4):
                        dy, dx = dydx // 2, dydx % 2
                        rhs = bass.AP(
                            tensor=xa.tensor,
                            offset=xa.offset + s0 * 36 + dy * 6 + dx,
                            ap=[[xa.ap[0][0], 17], [36, 16], [6, 5], [1, 5]],
                        )
                        nc.tensor.matmul(pscv, wconv_s[:, dydx], rhs,
                                         start=(dydx == 0), stop=(dydx == 3))
                    pv = pscv.rearrange("p (b i) -> p b i", i=25)
                    gp = slice(32 * g, 32 * g + 32)
                    bp = slice(bp0, bp0 + 16)
                    nc.scalar.copy(q_d[gp, bp, :, 0:1], pv[0:32].unsqueeze(3))
                    nc.scalar.copy(k_t[gp, bp, 0:25], pv[32:64])
                    nc.scalar.copy(v_t[gp, bp, 0:25], pv[64:96])
                    nc.scalar.activation(y_t[0:32, s0:s0 + 16], pv[96:128],
                                         AF.Relu)
            # duplicate q into pair slot 1 (DVE, 2x_2p)
            nc.vector.tensor_copy(q_d[:, :, :, 1:2], q_d[:, :, :, 0:1])

            # ---- attention ----
            attn_t = att_p.tile([128, BP, 25], f16, tag="attn_t")
            for s in range(NSUB):
                sl = slice(SB * s, SB * (s + 1))
                E = E_b[s % 2]
                Ev = Ev_b[s % 2]
                T1n = T1n_b[s % 2]
                T1d = T1d_b[s % 2]
                z = z_p.tile([128, SB, 25, 26], f16)
                q_bc = bass.AP(tensor=q_d.tensor, offset=q_d.offset + SB * s * 50,
                               ap=[q_d.ap[0], [50, SB], [2, 25], [0, 13], [1, 2]])
                k_bc = bass.AP(tensor=k_t.tensor, offset=k_t.offset + SB * s * 26,
                               ap=[k_t.ap[0], [26, SB], [0, 25], [1, 26]])
                v_bc = bass.AP(tensor=v_t.tensor, offset=v_t.offset + SB * s * 26,
                               ap=[v_t.ap[0], [26, SB], [0, 25], [1, 25]])
                nc.vector.tensor_mul(z, q_bc, k_bc)
                nc.scalar.activation(E[:, :, :, 0:25], z[:, :, :, 0:25],
                                     AF.Exp, bias=expb_s, scale=1.0)
                nc.vector.tensor_mul(Ev[:, :, :, 0:25], E[:, :, :, 0:25], v_bc)
                # two-stage pair-adds (2x TT) + 7-wide reduce
                nc.vector.tensor_add(T1n[:, :, :, 0:14], Ev[:, :, :, 0:14],
                                     Ev[:, :, :, 14:28])
                nc.vector.tensor_add(T1d[:, :, :, 0:14], E[:, :, :, 0:14],
                                     E[:, :, :, 14:28])
                T2n = t2_p.tile([128, SB, 25, 8], f16, tag="T2n")
                T2d = t2_p.tile([128, SB, 25, 8], f16, tag="T2d")
                nc.vector.tensor_add(T2n, T1n[:, :, :, 0:8], T1n[:, :, :, 8:16])
                nc.vector.tensor_add(T2d, T1d[:, :, :, 0:8], T1d[:, :, :, 8:16])
                num = red_p.tile([128, SB, 25], f32, tag="num")
                den = red_p.tile([128, SB, 25], f32, tag="den")
                nc.vector.tensor_reduce(num, T2n, axis=AX.X, op=ALU.add)
                nc.vector.tensor_reduce(den, T2d, axis=AX.X, op=ALU.add)
                rden = red_p.tile([128, SB, 25], f32, tag="rden")
                nc.vector.reciprocal_approx_fast(rden, den)
                nc.gpsimd.tensor_mul(attn_t[:, sl, :], num, rden)

            # ---- 1x1 conv (block-diag stationary) + relu -> y_t rows 32:64 ----
            atf = attn_t.rearrange("p b i -> p (b i)")
            ytf = y_t.rearrange("p b i -> p (b i)")
            for t0 in range(2):
                psat = ps_att.tile([128, 400], f32)
                nc.tensor.matmul(psat, aw_s, atf[:, 400 * t0:400 * t0 + 400],
                                 start=True, stop=True)
                for g in range(4):
                    nc.scalar.activation(
                        ytf[32:64, 800 * g + 400 * t0:800 * g + 400 * t0 + 400],
                        psat[32 * g:32 * g + 32], AF.Relu, bias=ab_s, scale=1.0)

            # ---- dense1: accumulate over 25 pixels ----
            ps1 = ps_mlp.tile([128, CB], f32, tag="ps1")
            for i in range(25):
                nc.tensor.matmul(ps1, w1_s[:, :, i], y_t[:, :, i],
                                 start=(i == 0), stop=(i == 24))
            y1 = mlp_p.tile([128, CB], f16, tag="y1")
            nc.scalar.activation(y1, ps1, AF.Relu, bias=b1_s, scale=1.0)

            # ---- dense2 ----
            ps2 = ps_mlp.tile([64, CB], f32, tag="ps2")
            nc.tensor.matmul(ps2, w2_s, y1, start=True, stop=True)
            y2 = mlp_p.tile([64, CB], f16, tag="y2")
            nc.scalar.activation(y2, ps2, AF.Relu, bias=b2_s, scale=1.0)

            # ---- dense3 (bias added on host) ----
            ps3 = ps_mlp.tile([5, CB], f32, tag="ps3")
            nc.tensor.matmul(ps3, w3_s, y2, start=True, stop=True)
            outs = outp_p.tile([5, CB], f32)
            nc.scalar.copy(outs, ps3)
            nc.sync.dma_start(
                out=bass.AP(tensor=out_d, offset=b0, ap=[[BL, 5], [1, CB]]),
                in_=outs)

    nc.finalize()
    return nc, in_names, out_d.name


_PROG = None


def _get_program():
    global _PROG
    if _PROG is None:
        _PROG = _build_program()
    return _PROG


def _host_conv(x, w, b):
    """2x2 VALID conv, NCHW, numpy. Returns [B, O, 25] float32."""
    B_, C_, H_, W_ = x.shape
    out = None
    for dy in range(2):
        for dx in range(2):
            xs = x[:, :, dy:dy + 5, dx:dx + 5].reshape(B_, C_, 25)
            t = np.einsum('oc,bcp->bop', w[:, :, dy, dx], xs,
                          optimize=True)
            out = t if out is None else out + t
    return (out + b[None, :, None]).astype(np.float32)


def _make_in_maps(inputs):
    return _host_prep(**inputs)


def _host_prep(x, conv_w, conv_b, qkv_w, qkv_b, attn_w, attn_b,
               w1, b1, w2, b2, w3, b3):
    # channel order [q|k|v|conv_out]; v rows pre-scaled by 1/VS
    wc = np.concatenate([np.asarray(qkv_w), np.asarray(conv_w)], axis=0)  # [128,16,2,2]
    wc = wc.copy()
    wc[64:96] /= VS
    cb = np.concatenate([np.asarray(qkv_b), np.asarray(conv_b)]).astype(np.float64)
    cb = cb.copy()
    cb[64:96] /= VS
    # wconv17[c17, (2dy+dx), ch]: rows 0:16 weights, row 16 bias/4
    wconv = np.zeros((17, 4, 128), np.float16)
    wconv[0:16] = wc.transpose(1, 2, 3, 0).reshape(16, 4, 128).astype(np.float16)
    wconv[16] = (cb / 4.0)[None, :].astype(np.float16)

    # block-diag 1x1 conv stationary [(g,h), (g,c)] = attn_w[c, h] * VS
    aw = np.asarray(attn_w)[:, :, 0, 0].astype(np.float32) * VS   # [c32, h32]
    aw_rep = np.zeros((128, 128), np.float16)
    for g in range(4):
        aw_rep[32 * g:32 * g + 32, 32 * g:32 * g + 32] = aw.T.astype(np.float16)
    ab = np.asarray(attn_b)[:, None].astype(np.float32)

    w1t = np.ascontiguousarray(
        np.asarray(w1).reshape(HID, 64, 25).transpose(1, 0, 2)).astype(np.float16)
    b1c = np.asarray(b1)[:, None].astype(np.float32)
    w2t = np.ascontiguousarray(np.asarray(w2).T).astype(np.float16)
    b2c = np.asarray(b2)[:, None].astype(np.float32)
    w3t = np.ascontiguousarray(np.asarray(w3).T).astype(np.float16)

    x = np.asarray(x, dtype=np.float32)

    # exp-overflow guard: softmax invariant to exp(z - C0); C0 from the exact
    # global max of q_i*k_j (corner products of per-row min/max).
    qw, kw = np.asarray(qkv_w)[0:32], np.asarray(qkv_w)[32:64]
    qb_, kb_ = np.asarray(qkv_b)[0:32], np.asarray(qkv_b)[32:64]
    qv = _host_conv(x, qw, qb_)        # [B, 32, 25]
    kv = _host_conv(x, kw, kb_)
    qmax, qmin = qv.max(2), qv.min(2)  # [B, 32]
    kmax, kmin = kv.max(2), kv.min(2)
    zmax = max((qmax * kmax).max(), (qmax * kmin).max(),
               (qmin * kmax).max(), (qmin * kmin).max())
    c0 = float(max(0.0, zmax - 8.8))
    expb = np.full((128, 1), -c0, dtype=np.float32)

    wp32 = np.zeros((128, 4), np.float32)
    wp32[:, 0:1] = expb
    wp32[:, 1:2] = b1c
    wp32[0:64, 2:3] = b2c
    wp32[0:32, 3:4] = ab
    wp16 = np.zeros((128, 4000), np.float16)
    wp16[:, 0:64] = w2t
    wp16[0:64, 64:3264] = w1t.reshape(64, 3200)
    wp16[:, 3264:3392] = aw_rep
    wp16[0:64, 3392:3397] = w3t
    wp16[0:17, 3397:3909] = wconv.reshape(17, 512)

    x16 = x.astype(np.float16)
    shared = {"wp32": wp32, "wp16": wp16}
    in_maps = []
    for c in range(NCORES):
        m = dict(shared)
        m["x_s"] = np.ascontiguousarray(x16[c * BL:(c + 1) * BL])
        in_maps.append(m)
    return in_maps


def kernel(x, conv_w, conv_b, qkv_w, qkv_b, attn_w, attn_b,
           w1, b1, w2, b2, w3, b3):
    from concourse.bass_utils import run_bass_kernel_spmd

    nc, in_names, out_name = _get_program()
    in_maps = _host_prep(x, conv_w, conv_b, qkv_w, qkv_b, attn_w, attn_b,
                         w1, b1, w2, b2, w3, b3)
    res = run_bass_kernel_spmd(nc, in_maps, core_ids=list(range(NCORES)))
    outs = [r[out_name] for r in res.results]           # each [5, BL]
    full = np.concatenate([o.T for o in outs], axis=0)  # [8192, 5]
    full = full + np.asarray(b3)[None, :].astype(np.float32)
    return full.astype(np.float32)


# revision 10
# speedup vs baseline: 1.4006x; 1.4006x over previous
"""Trainium2 Bass kernel for nn_CNN2DAttWind_NL (conv + 32-head rank-1 attention + MLP).

Contract: kernel(**inputs) takes FULL unsharded numpy inputs, returns FULL output
[8192, 5] float32.  Internally shards the batch across 8 NeuronCores (pure data
parallel) and runs a Bass/Tile kernel via run_bass_kernel_spmd.

Layout strategy per core (B_local = 1024), chunk CB = 128 samples:
  - conv as matmul with the BIAS FOLDED IN as a 17th K-row (moving row of
    ones); run per b-quarter g so the eviction writes q/k/v straight into the
    pair-major attention layout [(g, head), b', pix] (no SBUF shuffle DMAs).
  - attention: partitions = (b-quarter, head) = 128 rows; free = (b', i, j).
    z = q*k on DVE/Pool (alternating subs), exp on ACT, Ev on DVE, then
    two-stage pair-adds (TT at 2x mode) + a 7-wide reduce for num/den
    (tensor_reduce is 1 elem/cycle, so feeding it 7/25th of the data wins).
  - 1x1 conv as a block-diagonal [128,128] stationary consuming attn in
    pair-major directly; 3-layer MLP as PE matmuls.
"""

import os
from contextlib import ExitStack

import numpy as np

# Problem constants (hardcoded per harness contract)
B, CIN, HIN, WIN = 8192, 16, 6, 6
FM = 64
DK = FM // 2          # 32
DV = FM // 2          # 32
NH = FM // 2          # 32 heads, dkh = dvh = 1
OUT_CH = 5
HID = 128
HW = 25               # 5x5 output pixels

NCORES = 8
BL = B // NCORES      # 1024 samples per core
CB = 128              # samples per outer chunk
NCHUNK = BL // CB     # 8
BP = CB // 4          # 32 b' per quarter
SB = 16               # b' groups per attention sub-chunk
NSUB = BP // SB       # 4
VS = 16.0             # v pre-scale to keep Ev pair-adds inside f16 range


def _build_program():
    import concourse.bass as bass
    import concourse.tile as tile
    from concourse import bacc, mybir

    f32 = mybir.dt.float32
    f16 = mybir.dt.float16
    AF = mybir.ActivationFunctionType
    ALU = mybir.AluOpType
    AX = mybir.AxisListType

    nc = bacc.Bacc(
        "TRN2",
        target_bir_lowering=False,
        debug=False,
        enable_asserts=False,
        num_devices=NCORES,
    )

    # ---- DRAM I/O ----
    x_d = nc.dram_tensor("x_s", [BL, CIN, HIN, WIN], f16, kind="ExternalInput")
    wp32_d = nc.dram_tensor("wp32", [128, 4], f32, kind="ExternalInput")
    wp16_d = nc.dram_tensor("wp16", [128, 4000], f16, kind="ExternalInput")
    out_d = nc.dram_tensor("out", [5, BL], f32, kind="ExternalOutput")

    in_names = [t.name for t in (x_d, wp32_d, wp16_d)]

    with tile.TileContext(nc) as tc, ExitStack() as ctx, \
            nc.allow_low_precision(reason="f16 softmax intermediates, 2e-2 budget"):
        singles = ctx.enter_context(tc.tile_pool(name="singles", bufs=1))
        xa_p = ctx.enter_context(tc.tile_pool(name="xa", bufs=2))
        qkv_p = ctx.enter_context(tc.tile_pool(name="qkv", bufs=2))
        yt_p = ctx.enter_context(tc.tile_pool(name="yt", bufs=2))
        att_p = ctx.enter_context(tc.tile_pool(name="att", bufs=2))
        z_p = ctx.enter_context(tc.tile_pool(name="z", bufs=2))
        t2_p = ctx.enter_context(tc.tile_pool(name="t2", bufs=1))
        red_p = ctx.enter_context(tc.tile_pool(name="red", bufs=1))
        mlp_p = ctx.enter_context(tc.tile_pool(name="mlp", bufs=2))
        outp_p = ctx.enter_context(tc.tile_pool(name="outp", bufs=2))
        ps_conv = ctx.enter_context(tc.tile_pool(name="psc", bufs=3, space="PSUM"))
        ps_att = ctx.enter_context(tc.tile_pool(name="psa", bufs=2, space="PSUM"))
        ps_mlp = ctx.enter_context(tc.tile_pool(name="psm", bufs=1, space="PSUM"))

        # ---- load all weights in 2 DMAs ----
        wp32 = singles.tile([128, 4], f32)
        nc.sync.dma_start(out=wp32, in_=wp32_d.ap())
        wp16 = singles.tile([128, 4000], f16)
        nc.sync.dma_start(out=wp16, in_=wp16_d.ap())
        expb_s = wp32[:, 0:1]
        b1_s = wp32[:, 1:2]
        b2_s = wp32[0:64, 2:3]
        ab_s = wp32[0:32, 3:4]
        w2_s = wp16[:, 0:64]
        w1_s = wp16[0:64, 64:3264].rearrange("p (h i) -> p h i", i=25)
        aw_s = wp16[:, 3264:3392]                # block-diag 1x1 conv [128,128]
        w3_s = wp16[0:64, 3392:3397]
        wconv_s = wp16[0:17, 3397:3909].rearrange("p (g c) -> p g c", g=4)

        # ---- persistent attention tiles (pad columns pre-zeroed once) ----
        # E/Ev are 26 wide: cols 0:25 written each use, col 25 stays 0.
        # T1 are 14 wide: cols 0:13 written each use, col 13 stays 0.
        E_b = [singles.tile([128, SB, 25, 28], f16, name=f"E{i}") for i in range(2)]
        Ev_b = [singles.tile([128, SB, 25, 28], f16, name=f"Ev{i}") for i in range(1)]
        T1n_b = [singles.tile([128, SB, 25, 16], f16, name=f"T1n{i}") for i in range(1)]
        T1d_b = [singles.tile([128, SB, 25, 16], f16, name=f"T1d{i}") for i in range(1)]
        for t in E_b + Ev_b:
            nc.vector.memset(t[:, :, :, 25:28], 0.0)
        for t in T1n_b + T1d_b:
            nc.vector.memset(t[:, :, :, 14:16], 0.0)
        # xa double buffer: rows 0:16 = x, row 16 = ones (bias path)
        xa_b = [singles.tile([32, CB, 36], f16, name=f"xa{i}") for i in range(2)]
        for t in xa_b:
            nc.vector.memset(t, 1.0)

        for c in range(NCHUNK):
            b0 = c * CB
            xa = xa_b[c % 2]
            src = bass.AP(tensor=x_d, offset=b0 * 576,
                          ap=[[36, 16], [576, CB], [1, 36]])
            nc.sync.dma_start(out=xa[0:16], in_=src)

            # ---- conv per quarter: psum rows = q(0:32) k v conv_out ----
            q_d = qkv_p.tile([128, BP, 25, 2], f16, tag="q_d")
            k_t = qkv_p.tile([128, BP, 26], f16, tag="k_t")
            v_t = qkv_p.tile([128, BP, 26], f16, tag="v_t")
            y_t = yt_p.tile([64, CB, 25], f16, tag="y_t")
            for g in range(4):
                for t0 in range(2):             # two 400-col tiles of 16 b'
                    s0 = 32 * g + 16 * t0       # first sample of tile
                    bp0 = 16 * t0               # first b' of tile
                    pscv = ps_conv.tile([128, 400], f32)
                    for dydx in range(4):
                        dy, dx = dydx // 2, dydx % 2
                        rhs = bass.AP(
                            tensor=xa.tensor,
                            offset=xa.offset + s0 * 36 + dy * 6 + dx,
                            ap=[[xa.ap[0][0], 17], [36, 16], [6, 5], [1, 5]],
                        )
                        nc.tensor.matmul(pscv, wconv_s[:, dydx], rhs,
                                         start=(dydx == 0), stop=(dydx == 3))
                    pv = pscv.rearrange("p (b i) -> p b i", i=25)
                    gp = slice(32 * g, 32 * g + 32)
                    bp = slice(bp0, bp0 + 16)
                    nc.scalar.copy(q_d[gp, bp, :, 0:1], pv[0:32].unsqueeze(3))
                    nc.scalar.copy(k_t[gp, bp, 0:25], pv[32:64])
                    nc.scalar.copy(v_t[gp, bp, 0:25], pv[64:96])
                    nc.scalar.activation(y_t[0:32, s0:s0 + 16], pv[96:128],
                                         AF.Relu)
            # duplicate q into pair slot 1 (DVE, 2x_2p)
            nc.vector.tensor_copy(q_d[:, :, :, 1:2], q_d[:, :, :, 0:1])

            # ---- attention ----
            attn_t = att_p.tile([128, BP, 25], f16, tag="attn_t")
            for s in range(NSUB):
                sl = slice(SB * s, SB * (s + 1))
                E = E_b[s % 2]
                Ev = Ev_b[0]
                T1n = T1n_b[0]
                T1d = T1d_b[0]
                z = z_p.tile([128, SB, 25, 26], f16)
                q_bc = bass.AP(tensor=q_d.tensor, offset=q_d.offset + SB * s * 50,
                               ap=[q_d.ap[0], [50, SB], [2, 25], [0, 13], [1, 2]])
                k_bc = bass.AP(tensor=k_t.tensor, offset=k_t.offset + SB * s * 26,
                               ap=[k_t.ap[0], [26, SB], [0, 25], [1, 26]])
                v_bc = bass.AP(tensor=v_t.tensor, offset=v_t.offset + SB * s * 26,
                               ap=[v_t.ap[0], [26, SB], [0, 25], [1, 25]])
                nc.vector.tensor_mul(z, q_bc, k_bc)
                nc.scalar.activation(E[:, :, :, 0:25], z[:, :, :, 0:25],
                                     AF.Exp, bias=expb_s, scale=1.0)
                nc.vector.tensor_mul(Ev[:, :, :, 0:25], E[:, :, :, 0:25], v_bc)
                # two-stage pair-adds (2x TT) + 7-wide reduce
                nc.vector.tensor_add(T1n[:, :, :, 0:14], Ev[:, :, :, 0:14],
                                     Ev[:, :, :, 14:28])
                nc.vector.tensor_add(T1d[:, :, :, 0:14], E[:, :, :, 0:14],
                                     E[:, :, :, 14:28])
                T2n = t2_p.tile([128, SB, 25, 8], f16, tag="T2n")
                T2d = t2_p.tile([128, SB, 25, 8], f16, tag="T2d")
                nc.vector.tensor_add(T2n, T1n[:, :, :, 0:8], T1n[:, :, :, 8:16])
                nc.vector.tensor_add(T2d, T1d[:, :, :, 0:8], T1d[:, :, :, 8:16])
                num = red_p.tile([128, SB, 25], f32, tag="num")
                den = red_p.tile([128, SB, 25], f32, tag="den")
                nc.vector.tensor_reduce(num, T2n, axis=AX.X, op=ALU.add)
                nc.vector.tensor_reduce(den, T2d, axis=AX.X, op=ALU.add)
                rden = red_p.tile([128, SB, 25], f32, tag="rden")
                nc.vector.reciprocal_approx_fast(rden, den)
                nc.vector.tensor_mul(attn_t[:, sl, :], num, rden)

            # ---- 1x1 conv (block-diag stationary) + relu -> y_t rows 32:64 ----
            atf = attn_t.rearrange("p b i -> p (b i)")
            ytf = y_t.rearrange("p b i -> p (b i)")
            for t0 in range(2):
                psat = ps_att.tile([128, 400], f32)
                nc.tensor.matmul(psat, aw_s, atf[:, 400 * t0:400 * t0 + 400],
                                 start=True, stop=True)
                for g in range(4):
                    nc.scalar.activation(
                        ytf[32:64, 800 * g + 400 * t0:800 * g + 400 * t0 + 400],
                        psat[32 * g:32 * g + 32], AF.Relu, bias=ab_s, scale=1.0)

            # ---- dense1: accumulate over 25 pixels ----
            ps1 = ps_mlp.tile([128, CB], f32, tag="ps1")
            for i in range(25):
                nc.tensor.matmul(ps1, w1_s[:, :, i], y_t[:, :, i],
                                 start=(i == 0), stop=(i == 24))
            y1 = mlp_p.tile([128, CB], f16, tag="y1")
            nc.scalar.activation(y1, ps1, AF.Relu, bias=b1_s, scale=1.0)

            # ---- dense2 ----
            ps2 = ps_mlp.tile([64, CB], f32, tag="ps2")
            nc.tensor.matmul(ps2, w2_s, y1, start=True, stop=True)
            y2 = mlp_p.tile([64, CB], f16, tag="y2")
            nc.scalar.activation(y2, ps2, AF.Relu, bias=b2_s, scale=1.0)

            # ---- dense3 (bias added on host) ----
            ps3 = ps_mlp.tile([5, CB], f32, tag="ps3")
            nc.tensor.matmul(ps3, w3_s, y2, start=True, stop=True)
            outs = outp_p.tile([5, CB], f32)
            nc.scalar.copy(outs, ps3)
            nc.sync.dma_start(
                out=bass.AP(tensor=out_d, offset=b0, ap=[[BL, 5], [1, CB]]),
                in_=outs)

    nc.finalize()
    return nc, in_names, out_d.name


_PROG = None


def _get_program():
    global _PROG
    if _PROG is None:
        _PROG = _build_program()
    return _PROG


def _host_conv(x, w, b):
    """2x2 VALID conv, NCHW, numpy. Returns [B, O, 25] float32."""
    B_, C_, H_, W_ = x.shape
    out = None
    for dy in range(2):
        for dx in range(2):
            xs = x[:, :, dy:dy + 5, dx:dx + 5].reshape(B_, C_, 25)
            t = np.einsum('oc,bcp->bop', w[:, :, dy, dx], xs,
                          optimize=True)
            out = t if out is None else out + t
    return (out + b[None, :, None]).astype(np.float32)


def _make_in_maps(inputs):
    return _host_prep(**inputs)


def _host_prep(x, conv_w, conv_b, qkv_w, qkv_b, attn_w, attn_b,
               w1, b1, w2, b2, w3, b3):
    # channel order [q|k|v|conv_out]; v rows pre-scaled by 1/VS
    wc = np.concatenate([np.asarray(qkv_w), np.asarray(conv_w)], axis=0)  # [128,16,2,2]
    wc = wc.copy()
    wc[64:96] /= VS
    cb = np.concatenate([np.asarray(qkv_b), np.asarray(conv_b)]).astype(np.float64)
    cb = cb.copy()
    cb[64:96] /= VS
    # wconv17[c17, (2dy+dx), ch]: rows 0:16 weights, row 16 bias/4
    wconv = np.zeros((17, 4, 128), np.float16)
    wconv[0:16] = wc.transpose(1, 2, 3, 0).reshape(16, 4, 128).astype(np.float16)
    wconv[16] = (cb / 4.0)[None, :].astype(np.float16)

    # block-diag 1x1 conv stationary [(g,h), (g,c)] = attn_w[c, h] * VS
    aw = np.asarray(attn_w)[:, :, 0, 0].astype(np.float32) * VS   # [c32, h32]
    aw_rep = np.zeros((128, 128), np.float16)
    for g in range(4):
        aw_rep[32 * g:32 * g + 32, 32 * g:32 * g + 32] = aw.T.astype(np.float16)
    ab = np.asarray(attn_b)[:, None].astype(np.float32)

    w1t = np.ascontiguousarray(
        np.asarray(w1).reshape(HID, 64, 25).transpose(1, 0, 2)).astype(np.float16)
    b1c = np.asarray(b1)[:, None].astype(np.float32)
    w2t = np.ascontiguousarray(np.asarray(w2).T).astype(np.float16)
    b2c = np.asarray(b2)[:, None].astype(np.float32)
    w3t = np.ascontiguousarray(np.asarray(w3).T).astype(np.float16)

    x = np.asarray(x, dtype=np.float32)

    # exp-overflow guard: softmax invariant to exp(z - C0); C0 from the exact
    # global max of q_i*k_j (corner products of per-row min/max).
    qw, kw = np.asarray(qkv_w)[0:32], np.asarray(qkv_w)[32:64]
    qb_, kb_ = np.asarray(qkv_b)[0:32], np.asarray(qkv_b)[32:64]
    qv = _host_conv(x, qw, qb_)        # [B, 32, 25]
    kv = _host_conv(x, kw, kb_)
    qmax, qmin = qv.max(2), qv.min(2)  # [B, 32]
    kmax, kmin = kv.max(2), kv.min(2)
    zmax = max((qmax * kmax).max(), (qmax * kmin).max(),
               (qmin * kmax).max(), (qmin * kmin).max())
    c0 = float(max(0.0, zmax - 8.8))
    expb = np.full((128, 1), -c0, dtype=np.float32)

    wp32 = np.zeros((128, 4), np.float32)
    wp32[:, 0:1] = expb
    wp32[:, 1:2] = b1c
    wp32[0:64, 2:3] = b2c
    wp32[0:32, 3:4] = ab
    wp16 = np.zeros((128, 4000), np.float16)
    wp16[:, 0:64] = w2t
    wp16[0:64, 64:3264] = w1t.reshape(64, 3200)
    wp16[:, 3264:3392] = aw_rep
    wp16[0:64, 3392:3397] = w3t
    wp16[0:17, 3397:3909] = wconv.reshape(17, 512)

    x16 = x.astype(np.float16)
    shared = {"wp32": wp32, "wp16": wp16}
    in_maps = []
    for c in range(NCORES):
        m = dict(shared)
        m["x_s"] = np.ascontiguousarray(x16[c * BL:(c + 1) * BL])
        in_maps.append(m)
    return in_maps


def kernel(x, conv_w, conv_b, qkv_w, qkv_b, attn_w, attn_b,
           w1, b1, w2, b2, w3, b3):
    from concourse.bass_utils import run_bass_kernel_spmd

    nc, in_names, out_name = _get_program()
    in_maps = _host_prep(x, conv_w, conv_b, qkv_w, qkv_b, attn_w, attn_b,
                         w1, b1, w2, b2, w3, b3)
    res = run_bass_kernel_spmd(nc, in_maps, core_ids=list(range(NCORES)))
    outs = [r[out_name] for r in res.results]           # each [5, BL]
    full = np.concatenate([o.T for o in outs], axis=0)  # [8192, 5]
    full = full + np.asarray(b3)[None, :].astype(np.float32)
    return full.astype(np.float32)
